# revision 31
# baseline (speedup 1.0000x reference)
"""Multi-head attention Trainium2 kernel (bs=4, slen=1024, dim=1024, 16 heads).

Sharding: 8 cores = 4 batches x 2 head-groups (8 heads / 512 features each).

v3: software-pipelined emission so the ScalarE exp stream hides under tensor
work and the PE never idles (keeps the HAM clock-gate warm at 2.4 GHz):

  PRE : q/k projection for head-pair 0 (feature-major, bias added on VectorE)
  A   : scores+exp(hp0) | v projection (seq-major) | q/k proj(hp1)
  B   : scores+exp(hp1) | ctx(hp0) + normalize     | q/k proj(hp2)
  C   : scores+exp(hp2) | ctx(hp1) + normalize     | q/k proj(hp3)
  D   : scores+exp(hp3) | ctx(hp2) + normalize
  TAIL: ctx(hp3), normalize, out-projection

Scores for the two heads of a pair go to one [128,2,512] PSUM tile so a
single ScalarE Exp (with the key-padding mask folded into the per-partition
bias, and the 1/sqrt(dh) fold into the activation scale) covers both heads.
ctxT rows carry an appended all-ones column so row 64 is the softmax
denominator; denominator rows stage through partition 0 (DVE cannot write
single rows at unaligned partition offsets) and DMA into a [2,512] tile per
(head-pair, qc), inverted with the fast approximate reciprocal and broadcast
back through a K=2 selector matmul. Input DMA issue is split across the
Sync and Scalar queues to halve the ~600ns-per-descriptor startup serial.
Host sums the two head-group partials per batch and adds out_b.
"""

import numpy as np

BS, SLEN, DIM = 4, 1024, 1024
H, DH = 16, 64
P = 128            # partitions
NB = 512           # matmul free-dim chunk (one PSUM bank of fp32)
FPC = 512          # features per core (8 heads)
DT = DIM // P      # 8 contraction tiles over model dim
FT = FPC // P      # 4 feature tiles per core (== head pairs)
QC = SLEN // NB    # 2 seq chunks
ST = SLEN // P     # 8 seq tiles
HP = 4             # head pairs per core

# matmul operand dtypes per stage: "f32r" or "bf16"
PROJ_DT = "bf16"
ATT_DT = "bf16"
OUT_DT = "bf16"

_STATE = {}

# set to True by test harness to capture an NTFF profile
TRACE = False
TRACE_KWARGS = {}
LAST_RESULT = None


def _np_dt(kind):
    if kind == "bf16":
        import ml_dtypes
        return ml_dtypes.bfloat16
    return np.float32


def _build():
    from contextlib import ExitStack

    import concourse.tile as tile
    from concourse import bacc, mybir

    f32 = mybir.dt.float32
    f32r = mybir.dt.float32r
    bf16 = mybir.dt.bfloat16
    AF = mybir.ActivationFunctionType

    dts = {"f32r": f32r, "bf16": bf16}
    pdt, adt, odt = dts[PROJ_DT], dts[ATT_DT], dts[OUT_DT]

    nc = bacc.Bacc("TRN2", target_bir_lowering=False, debug=False)

    xt_d = nc.dram_tensor("xt", [DIM, SLEN], pdt, kind="ExternalInput")
    wqt_d = nc.dram_tensor("wqt", [DIM, FPC], pdt, kind="ExternalInput")
    wkt_d = nc.dram_tensor("wkt", [DIM, FPC], pdt, kind="ExternalInput")
    wvt_d = nc.dram_tensor("wvt", [DIM, FPC], pdt, kind="ExternalInput")
    wot_d = nc.dram_tensor("wot", [FPC, DIM], odt, kind="ExternalInput")
    qb_d = nc.dram_tensor("qb", [P, FT], f32, kind="ExternalInput")
    kb_d = nc.dram_tensor("kb", [P, FT], f32, kind="ExternalInput")
    negb_d = nc.dram_tensor("negb", [P, ST], f32, kind="ExternalInput")
    sel_d = nc.dram_tensor("sel", [2, P], f32r, kind="ExternalInput")
    out_d = nc.dram_tensor("out", [SLEN, DIM], f32, kind="ExternalOutput")

    with tile.TileContext(nc) as tc:
        with ExitStack() as ctx:
            consts = ctx.enter_context(tc.tile_pool(name="consts", bufs=1))
            big = ctx.enter_context(tc.tile_pool(name="big", bufs=1))
            wtsp = ctx.enter_context(tc.tile_pool(name="wtsp", bufs=4))
            sm = ctx.enter_context(tc.tile_pool(name="sm", bufs=4))
            psum = ctx.enter_context(tc.tile_pool(name="psum", bufs=1, space="PSUM"))

            # ---- inputs, all on the Sync queue in priority order: the
            # ~0.65us-per-descriptor issue serialization doubles as a
            # bandwidth priority (critical-path tensors first). Scalar
            # carries only the warmup exp (hoists the ACT table load). ----
            ones_f = consts.tile([1, P], f32)
            nc.vector.memset(ones_f, 1.0)
            warm = consts.tile([1, P], f32)
            nc.scalar.activation(warm, ones_f, AF.Exp, scale=1.0)

            wqt_sb = big.tile([P, DT, FPC], pdt, tag="wq")
            wkt_sb = big.tile([P, DT, FPC], pdt, tag="wk")
            wvt_sb = big.tile([P, DT, FPC], pdt, tag="wv")
            wot_sb = big.tile([P, FT, DIM], odt, tag="wo")
            xt_a = big.tile([P, DT // 2, SLEN], pdt, tag="xa")
            xt_b = big.tile([P, DT // 2, SLEN], pdt, tag="xb")

            def xts(t):
                xh = xt_a if t < DT // 2 else xt_b
                return xh[:, t % (DT // 2), :]

            def wcol(w_d, w_sb, ft):
                cs = slice(ft * P, (ft + 1) * P)
                nc.sync.dma_start(
                    w_sb[:, :, cs],
                    w_d[:, cs].rearrange("(t p) f -> p t f", p=P))

            def xchunk(half, qc):
                """One (t-half, qc) quadrant of x: the qc0 quadrants are all
                the q/k projections for query-chunk 0 need, so the pipeline
                starts ~4us earlier than waiting for the full 2MB of x."""
                xh = xt_a if half == 0 else xt_b
                rs = slice(half * (DIM // 2), (half + 1) * (DIM // 2))
                cs = slice(qc * NB, (qc + 1) * NB)
                nc.sync.dma_start(
                    xh[:, :, cs],
                    xt_d[rs, cs].rearrange("(t p) f -> p t f", p=P))

            wcol(wqt_d, wqt_sb, 0)
            xchunk(0, 0)
            xchunk(1, 0)
            qb_sb = consts.tile([P, FT], f32)
            nc.sync.dma_start(qb_sb, qb_d[:])
            kb_sb = consts.tile([P, FT], f32)
            nc.sync.dma_start(kb_sb, kb_d[:])
            wcol(wkt_d, wkt_sb, 0)
            xchunk(0, 1)
            xchunk(1, 1)
            negb_sb = consts.tile([P, ST], f32)
            nc.sync.dma_start(negb_sb, negb_d[:])
            nc.sync.dma_start(
                wvt_sb, wvt_d[:].rearrange("(t p) f -> p t f", p=P))
            wcol(wqt_d, wqt_sb, 1)
            for ft in range(1, FT):
                wcol(wkt_d, wkt_sb, ft)
            sel_sb = consts.tile([2, P], f32r)
            nc.sync.dma_start(sel_sb, sel_d[:])
            for ft in range(2, FT):
                wcol(wqt_d, wqt_sb, ft)
            nc.sync.dma_start(
                wot_sb, wot_d[:].rearrange("(t p) f -> p t f", p=P))

            # ---- persistent activations ----
            qT_sb = big.tile([P, FT, SLEN], adt, tag="qT")   # [f%128, ft, seq]
            kT_sb = big.tile([P, FT, SLEN], adt, tag="kT")
            v_sb = big.tile([P, ST, HP * 2, DH + 1], adt, tag="v")
            ctall = big.tile([P, HP * QC, NB], f32, tag="ct")  # unnormalized ctxT
            ctxn_sb = big.tile([P, HP, SLEN], odt, tag="cn")   # normalized ctx.T
            vones_f = consts.tile([P, ST, HP * 2, 1], f32)
            nc.vector.memset(vones_f, 1.0)
            nc.vector.tensor_copy(v_sb[:, :, :, DH:DH + 1], vones_f)

            rca = {}  # (hp, qc) -> [2, NB] f32 reciprocal-denominator tile

            # ---- emission helpers (pipeline stages) ----
            def qkproj_group(ft, qc, which):
                """One 8-deep accumulation group of q or k projection,
                drained on VectorE with the bias add."""
                sl = slice(qc * NB, (qc + 1) * NB)
                w_sb = wqt_sb if which == 0 else wkt_sb
                dst = qT_sb if which == 0 else kT_sb
                b_sb = qb_sb if which == 0 else kb_sb
                ps = psum.tile([P, NB], f32, tag="aux", bufs=2, name="ps_p")
                for t in range(DT):
                    nc.tensor.matmul(
                        ps, lhsT=w_sb[:, t, ft * P:(ft + 1) * P],
                        rhs=xts(t)[:, sl],
                        start=(t == 0), stop=(t == DT - 1))
                nc.vector.tensor_scalar_add(
                    dst[:, ft, sl], ps, b_sb[:, ft:ft + 1])

            def vproj_group(st):
                """v projection for one seq tile (all 8 heads), drained on
                VectorE. v_b is NOT added here: softmax weights sum to 1, so
                its contribution is the constant v_b @ out_w.T, folded into
                out_b on the host."""
                ps_v = psum.tile([P, NB], f32, tag="aux", bufs=2, name="ps_v")
                for t in range(DT):
                    nc.tensor.matmul(
                        ps_v, lhsT=xts(t)[:, st * P:(st + 1) * P],
                        rhs=wvt_sb[:, t, :],
                        start=(t == 0), stop=(t == DT - 1))
                nc.vector.tensor_copy(
                    v_sb[:, st, :, 0:DH],
                    ps_v.rearrange("p (h e) -> p h e", h=HP * 2))

            def scores_pair(hp, qc, kt, wts_t):
                """scoresT for both heads of pair hp into one 2-bank PSUM
                tile; single merged Exp on ScalarE (scale=1/8, mask bias)."""
                sl = slice(qc * NB, (qc + 1) * NB)
                ksl = slice(kt * P, (kt + 1) * P)
                ps = psum.tile([P, 2, NB], f32, tag="s", bufs=2, name="ps_s")
                nc.tensor.matmul(
                    ps[:, 0, :], lhsT=kT_sb[0:DH, hp, ksl],
                    rhs=qT_sb[0:DH, hp, sl], tile_position=(0, 0))
                nc.tensor.matmul(
                    ps[:, 1, :], lhsT=kT_sb[DH:P, hp, ksl],
                    rhs=qT_sb[DH:P, hp, sl], tile_position=(DH, 0))
                nc.scalar.activation(
                    wts_t[:, kt, :, :], ps[:, :, :], AF.Exp,
                    bias=negb_sb[:, kt:kt + 1], scale=0.125)

            def ctx_pair(hp, qc, k2, pcA, pcB, wts_t):
                """ctxT accumulation for key tiles 2*k2, 2*k2+1."""
                for a in range(2):
                    kt = 2 * k2 + a
                    nc.tensor.matmul(
                        pcA, lhsT=v_sb[:, kt, 2 * hp, :],
                        rhs=wts_t[:, kt, 0, :],
                        start=(kt == 0), stop=(kt == ST - 1))
                    nc.tensor.matmul(
                        pcB, lhsT=v_sb[:, kt, 2 * hp + 1, :],
                        rhs=wts_t[:, kt, 1, :],
                        start=(kt == 0), stop=(kt == ST - 1))

            def ctx_drain(hp, qc, pcA, pcB, use_scalar=False):
                """ctxT + denominator rows out of PSUM, then the fast
                approximate reciprocal of the [2,512] denominator. When
                ScalarE is idle (exp stream finished), half the copies go
                there so the single-buffered ctx accumulators free sooner."""
                j = hp * QC + qc
                dnl2 = sm.tile([2, NB], f32, tag="dnl2", bufs=4, name="dnl2")
                for a, pc in ((0, pcA), (1, pcB)):
                    dtmp = sm.tile([1, NB], f32, tag="dtmp", bufs=4,
                                   name="dtmp")
                    if use_scalar and a == 1:
                        nc.scalar.copy(
                            ctall[a * DH:(a + 1) * DH, j, :], pc[0:DH, :])
                        nc.scalar.copy(dtmp, pc[DH:DH + 1, :])
                    else:
                        nc.vector.tensor_copy(
                            ctall[a * DH:(a + 1) * DH, j, :], pc[0:DH, :])
                        nc.vector.tensor_copy(dtmp, pc[DH:DH + 1, :])
                    nc.sync.dma_start(dnl2[a:a + 1, :], dtmp)
                # fast approx reciprocal (~18 bits), written as f32r so the
                # selector matmul can consume it on the fast PE path
                from concourse.dve_ops import (
                    RECIP_APPROX_FAST_CONSTS, RECIPROCAL_APPROX_FAST)
                r = sm.tile([2, NB], f32r, tag="rca2", bufs=4, name="rca2")
                c = RECIP_APPROX_FAST_CONSTS
                nc.vector._custom_dve(
                    RECIPROCAL_APPROX_FAST, out=r, in0=dnl2,
                    s0=c["s0"], s1=c["s1"], imm2=c["imm2"])
                rca[(hp, qc)] = r

            def normalize(hp, qc):
                j = hp * QC + qc
                sl = slice(qc * NB, (qc + 1) * NB)
                pb = psum.tile([P, NB], f32, tag="aux", bufs=2, name="pb")
                nc.tensor.matmul(pb, lhsT=sel_sb, rhs=rca[(hp, qc)])
                nc.vector.tensor_mul(ctxn_sb[:, hp, sl], ctall[:, j, :], pb)

            def outproj_qt(qt):
                """Out-projection for one 128-row seq tile (PSUM out is
                capped at one bank per matmul, so two 512-wide groups);
                drains alternate between ScalarE and VectorE."""
                po = psum.tile([P, 2, NB], f32, tag="s", bufs=2, name="po_s")
                ob2 = sm.tile([P, 2, NB], f32, tag="outsb", bufs=3,
                              name="ob2")
                for jc in range(QC):
                    for ft in range(FT):
                        nc.tensor.matmul(
                            po[:, jc, :],
                            lhsT=ctxn_sb[:, ft, qt * P:(qt + 1) * P],
                            rhs=wot_sb[:, ft, jc * NB:(jc + 1) * NB],
                            start=(ft == 0), stop=(ft == FT - 1))
                    if (qt + jc) % 2 == 0:
                        nc.scalar.copy(ob2[:, jc, :], po[:, jc, :])
                    else:
                        nc.vector.tensor_copy(ob2[:, jc, :], po[:, jc, :])
                    nc.sync.dma_start(
                        out_d[qt * P:(qt + 1) * P, jc * NB:(jc + 1) * NB],
                        ob2[:, jc, :])

            def wts_tile(name):
                return wtsp.tile([P, ST, 2, NB], adt, tag="wts", name=name)

            # ---- PRE: q/k projection for head pair 0. The first four
            # scores tiles (qc0, keys 0:511) slot in between the qc0 and qc1
            # projection groups: they only need the qc0 quadrants of x, so
            # they fill the DMA wait and start the exp stream early. ----
            wts_cur = {}
            pc_cur = {}
            with nc.named_scope("pre"):
                qkproj_group(0, 0, 0)
                qkproj_group(0, 0, 1)
                wts_cur[(0, 0)] = wts_tile("wts_0_0")
                for kt in range(4):
                    scores_pair(0, 0, kt, wts_cur[(0, 0)])
                qkproj_group(0, 1, 0)
                qkproj_group(0, 1, 1)

            # ---- steady-state pipeline over head pairs ----
            for step in range(HP):        # step = hp being scored
                with nc.named_scope(f"step{step}"):
                    for qc in range(QC):
                        if (step, qc) not in wts_cur:
                            wts_cur[(step, qc)] = wts_tile(
                                f"wts_{step}_{qc}")
                    for kt in range(ST):
                        if not (step == 0 and kt < 4):
                            scores_pair(step, 0, kt, wts_cur[(step, 0)])
                        scores_pair(step, 1, kt, wts_cur[(step, 1)])
                        if step == 0:
                            vproj_group(kt)
                        else:
                            hp = step - 1
                            qc = 0 if kt < 4 else 1
                            k2 = kt % 4
                            if k2 == 0:
                                pc_cur[0] = psum.tile(
                                    [DH + 1, NB], f32, tag="cA", bufs=1,
                                    name="pcA")
                                pc_cur[1] = psum.tile(
                                    [DH + 1, NB], f32, tag="cB", bufs=1,
                                    name="pcB")
                            ctx_pair(hp, qc, k2, pc_cur[0], pc_cur[1],
                                     wts_cur[(hp, qc)])
                            if k2 == 3:
                                ctx_drain(hp, qc, pc_cur[0], pc_cur[1])
                        if kt == 6 and step >= 1:
                            normalize(step - 1, 0)
                        if kt == 1 and step >= 2:
                            normalize(step - 2, 1)
                        # q/k proj for head pair step+1: 4 groups of 8.
                        # The last group of pair 3 (keys 512:1023) moves into
                        # step D, which is otherwise scalar-bound; scores of
                        # key tiles >= 4 only start at iteration 4.
                        if step < HP - 1 and kt % 2 == 0:
                            g = kt // 2
                            if not (step == HP - 2 and g == 3):
                                qkproj_group(step + 1, g // 2, g % 2)
                        if step == HP - 1 and kt == 0:
                            qkproj_group(HP - 1, 1, 1)

            # ---- tail: ctx(hp3) interleaved with out-projection ----
            with nc.named_scope("tail"):
                hp = HP - 1
                pcA = psum.tile([DH + 1, NB], f32, tag="cA", bufs=1,
                                name="pcA_t0")
                pcB = psum.tile([DH + 1, NB], f32, tag="cB", bufs=1,
                                name="pcB_t0")
                for k2 in range(4):
                    ctx_pair(hp, 0, k2, pcA, pcB, wts_cur[(hp, 0)])
                normalize(HP - 2, 1)   # pending from step D
                ctx_drain(hp, 0, pcA, pcB, use_scalar=True)
                pcA = psum.tile([DH + 1, NB], f32, tag="cA", bufs=1,
                                name="pcA_t1")
                pcB = psum.tile([DH + 1, NB], f32, tag="cB", bufs=1,
                                name="pcB_t1")
                ctx_pair(hp, 1, 0, pcA, pcB, wts_cur[(hp, 1)])
                ctx_pair(hp, 1, 1, pcA, pcB, wts_cur[(hp, 1)])
                normalize(hp, 0)
                outproj_qt(0)
                ctx_pair(hp, 1, 2, pcA, pcB, wts_cur[(hp, 1)])
                ctx_pair(hp, 1, 3, pcA, pcB, wts_cur[(hp, 1)])
                outproj_qt(1)
                ctx_drain(hp, 1, pcA, pcB, use_scalar=True)
                outproj_qt(2)
                outproj_qt(3)
                normalize(hp, 1)
                for qt in range(ST // QC, ST):
                    outproj_qt(qt)

    nc.compile()
    return nc


def _get_nc():
    if "nc" not in _STATE:
        _STATE["nc"] = _build()
    return _STATE["nc"]


def _sel_const():
    sel = np.zeros((2, P), np.float32)
    sel[0, 0:DH] = 1.0
    sel[1, DH:P] = 1.0
    return sel


def _in_maps(x, mask, q_w, q_b, k_w, k_b, v_w, v_b, out_w):
    f = np.float32
    pnp = _np_dt(PROJ_DT)
    onp = _np_dt(OUT_DT)
    maps = []
    for c in range(8):
        b, g = divmod(c, 2)
        fs = slice(g * FPC, (g + 1) * FPC)
        maps.append({
            "xt": np.ascontiguousarray(x[b].T).astype(pnp),
            "wqt": np.ascontiguousarray(q_w[fs, :].T).astype(pnp),
            "wkt": np.ascontiguousarray(k_w[fs, :].T).astype(pnp),
            "wvt": np.ascontiguousarray(v_w[fs, :].T).astype(pnp),
            "wot": np.ascontiguousarray(out_w[:, fs].T).astype(onp),
            "qb": np.ascontiguousarray(q_b[fs].astype(f).reshape(FT, P).T),
            "kb": np.ascontiguousarray(k_b[fs].astype(f).reshape(FT, P).T),
            "negb": np.ascontiguousarray(
                np.where(mask[b] == 0, f(-30000.0), f(0.0)).astype(f)
                .reshape(ST, P).T),
            "sel": _sel_const(),
        })
    return maps


def kernel(x, mask, q_w, q_b, k_w, k_b, v_w, v_b, out_w, out_b):
    global LAST_RESULT
    from concourse import bass_utils

    x = np.asarray(x, np.float32)
    mask = np.asarray(mask)
    nc = _get_nc()
    maps = _in_maps(x, mask, np.asarray(q_w, np.float32),
                    np.asarray(q_b, np.float32), np.asarray(k_w, np.float32),
                    np.asarray(k_b, np.float32), np.asarray(v_w, np.float32),
                    np.asarray(v_b, np.float32), np.asarray(out_w, np.float32))
    res = bass_utils.run_bass_kernel_spmd(
        nc, maps, core_ids=list(range(8)), trace=TRACE,
        trace_kwargs=TRACE_KWARGS)
    LAST_RESULT = res
    # v_b's contribution to the output is the constant v_b @ out_w.T
    # (softmax weights sum to 1), folded into the output bias here.
    out_b = np.asarray(out_b, np.float32) + (
        np.asarray(v_b, np.float32) @ np.asarray(out_w, np.float32).T)
    full = np.empty((BS, SLEN, DIM), np.float32)
    for b in range(BS):
        full[b] = res.results[2 * b]["out"] + res.results[2 * b + 1]["out"] + out_b
    return full


# revision 32
# speedup vs baseline: 1.0095x; 1.0095x over previous
"""Multi-head attention Trainium2 kernel (bs=4, slen=1024, dim=1024, 16 heads).

Sharding: 8 cores = 4 batches x 2 head-groups (8 heads / 512 features each).

v3: software-pipelined emission so the ScalarE exp stream hides under tensor
work and the PE never idles (keeps the HAM clock-gate warm at 2.4 GHz):

  PRE : q/k projection for head-pair 0 (feature-major, bias added on VectorE)
  A   : scores+exp(hp0) | v projection (seq-major) | q/k proj(hp1)
  B   : scores+exp(hp1) | ctx(hp0) + normalize     | q/k proj(hp2)
  C   : scores+exp(hp2) | ctx(hp1) + normalize     | q/k proj(hp3)
  D   : scores+exp(hp3) | ctx(hp2) + normalize
  TAIL: ctx(hp3), normalize, out-projection

Scores for the two heads of a pair go to one [128,2,512] PSUM tile so a
single ScalarE Exp (with the key-padding mask folded into the per-partition
bias, and the 1/sqrt(dh) fold into the activation scale) covers both heads.
ctxT rows carry an appended all-ones column so row 64 is the softmax
denominator; denominator rows stage through partition 0 (DVE cannot write
single rows at unaligned partition offsets) and DMA into a [2,512] tile per
(head-pair, qc), inverted with the fast approximate reciprocal and broadcast
back through a K=2 selector matmul. Input DMA issue is split across the
Sync and Scalar queues to halve the ~600ns-per-descriptor startup serial.
Host sums the two head-group partials per batch and adds out_b.
"""

import numpy as np

BS, SLEN, DIM = 4, 1024, 1024
H, DH = 16, 64
P = 128            # partitions
NB = 512           # matmul free-dim chunk (one PSUM bank of fp32)
FPC = 512          # features per core (8 heads)
DT = DIM // P      # 8 contraction tiles over model dim
FT = FPC // P      # 4 feature tiles per core (== head pairs)
QC = SLEN // NB    # 2 seq chunks
ST = SLEN // P     # 8 seq tiles
HP = 4             # head pairs per core

# matmul operand dtypes per stage: "f32r" or "bf16"
PROJ_DT = "bf16"
ATT_DT = "bf16"
OUT_DT = "bf16"

_STATE = {}

# set to True by test harness to capture an NTFF profile
TRACE = False
TRACE_KWARGS = {}
LAST_RESULT = None


def _np_dt(kind):
    if kind == "bf16":
        import ml_dtypes
        return ml_dtypes.bfloat16
    return np.float32


def _build():
    from contextlib import ExitStack

    import concourse.tile as tile
    from concourse import bacc, mybir

    f32 = mybir.dt.float32
    f32r = mybir.dt.float32r
    bf16 = mybir.dt.bfloat16
    AF = mybir.ActivationFunctionType

    dts = {"f32r": f32r, "bf16": bf16}
    pdt, adt, odt = dts[PROJ_DT], dts[ATT_DT], dts[OUT_DT]

    nc = bacc.Bacc("TRN2", target_bir_lowering=False, debug=False)

    xt_d = nc.dram_tensor("xt", [DIM, SLEN], pdt, kind="ExternalInput")
    wqt_d = nc.dram_tensor("wqt", [DIM, FPC], pdt, kind="ExternalInput")
    wkt_d = nc.dram_tensor("wkt", [DIM, FPC], pdt, kind="ExternalInput")
    wvt_d = nc.dram_tensor("wvt", [DIM, FPC], pdt, kind="ExternalInput")
    wot_d = nc.dram_tensor("wot", [FPC, DIM], odt, kind="ExternalInput")
    qb_d = nc.dram_tensor("qb", [P, FT], f32, kind="ExternalInput")
    kb_d = nc.dram_tensor("kb", [P, FT], f32, kind="ExternalInput")
    negb_d = nc.dram_tensor("negb", [P, ST], f32, kind="ExternalInput")
    sel_d = nc.dram_tensor("sel", [2, P], f32r, kind="ExternalInput")
    out_d = nc.dram_tensor("out", [SLEN, DIM], f32, kind="ExternalOutput")

    with tile.TileContext(nc) as tc:
        with ExitStack() as ctx:
            consts = ctx.enter_context(tc.tile_pool(name="consts", bufs=1))
            big = ctx.enter_context(tc.tile_pool(name="big", bufs=1))
            wtsp = ctx.enter_context(tc.tile_pool(name="wtsp", bufs=4))
            sm = ctx.enter_context(tc.tile_pool(name="sm", bufs=4))
            psum = ctx.enter_context(tc.tile_pool(name="psum", bufs=1, space="PSUM"))

            # ---- inputs, all on the Sync queue in priority order: the
            # ~0.65us-per-descriptor issue serialization doubles as a
            # bandwidth priority (critical-path tensors first). Scalar
            # carries only the warmup exp (hoists the ACT table load). ----
            ones_f = consts.tile([1, P], f32)
            nc.vector.memset(ones_f, 1.0)
            warm = consts.tile([1, P], f32)
            nc.scalar.activation(warm, ones_f, AF.Exp, scale=1.0)

            wqt_sb = big.tile([P, DT, FPC], pdt, tag="wq")
            wkt_sb = big.tile([P, DT, FPC], pdt, tag="wk")
            wvt_sb = big.tile([P, DT, FPC], pdt, tag="wv")
            wot_sb = big.tile([P, FT, DIM], odt, tag="wo")
            xt_a = big.tile([P, DT // 2, SLEN], pdt, tag="xa")
            xt_b = big.tile([P, DT // 2, SLEN], pdt, tag="xb")

            def xts(t):
                xh = xt_a if t < DT // 2 else xt_b
                return xh[:, t % (DT // 2), :]

            def wcol(w_d, w_sb, ft):
                cs = slice(ft * P, (ft + 1) * P)
                nc.sync.dma_start(
                    w_sb[:, :, cs],
                    w_d[:, cs].rearrange("(t p) f -> p t f", p=P))

            def xchunk(half, qc):
                """One (t-half, qc) quadrant of x: the qc0 quadrants are all
                the q/k projections for query-chunk 0 need, so the pipeline
                starts ~4us earlier than waiting for the full 2MB of x."""
                xh = xt_a if half == 0 else xt_b
                rs = slice(half * (DIM // 2), (half + 1) * (DIM // 2))
                cs = slice(qc * NB, (qc + 1) * NB)
                nc.sync.dma_start(
                    xh[:, :, cs],
                    xt_d[rs, cs].rearrange("(t p) f -> p t f", p=P))

            wcol(wqt_d, wqt_sb, 0)
            xchunk(0, 0)
            xchunk(1, 0)
            qb_sb = consts.tile([P, FT], f32)
            nc.sync.dma_start(qb_sb, qb_d[:])
            kb_sb = consts.tile([P, FT], f32)
            nc.sync.dma_start(kb_sb, kb_d[:])
            wcol(wkt_d, wkt_sb, 0)
            xchunk(0, 1)
            xchunk(1, 1)
            negb_sb = consts.tile([P, ST], f32)
            nc.sync.dma_start(negb_sb, negb_d[:])
            nc.sync.dma_start(
                wvt_sb, wvt_d[:].rearrange("(t p) f -> p t f", p=P))
            wcol(wqt_d, wqt_sb, 1)
            for ft in range(1, FT):
                wcol(wkt_d, wkt_sb, ft)
            sel_sb = consts.tile([2, P], f32r)
            nc.sync.dma_start(sel_sb, sel_d[:])
            for ft in range(2, FT):
                wcol(wqt_d, wqt_sb, ft)
            nc.sync.dma_start(
                wot_sb, wot_d[:].rearrange("(t p) f -> p t f", p=P))

            # ---- persistent activations ----
            qT_sb = big.tile([P, FT, SLEN], adt, tag="qT")   # [f%128, ft, seq]
            kT_sb = big.tile([P, FT, SLEN], adt, tag="kT")
            v_sb = big.tile([P, ST, HP * 2, DH + 1], adt, tag="v")
            ctall = big.tile([P, HP * QC, NB], f32, tag="ct")  # unnormalized ctxT
            ctxn_sb = big.tile([P, HP, SLEN], odt, tag="cn")   # normalized ctx.T
            vones_f = consts.tile([P, ST, HP * 2, 1], f32)
            nc.vector.memset(vones_f, 1.0)
            nc.vector.tensor_copy(v_sb[:, :, :, DH:DH + 1], vones_f)

            rca = {}  # (hp, qc) -> [2, NB] f32 reciprocal-denominator tile

            # ---- emission helpers (pipeline stages) ----
            def qkproj_group(ft, qc, which):
                """One 8-deep accumulation group of q or k projection,
                drained on VectorE with the bias add."""
                sl = slice(qc * NB, (qc + 1) * NB)
                w_sb = wqt_sb if which == 0 else wkt_sb
                dst = qT_sb if which == 0 else kT_sb
                b_sb = qb_sb if which == 0 else kb_sb
                ps = psum.tile([P, NB], f32, tag="aux", bufs=2, name="ps_p")
                for t in range(DT):
                    nc.tensor.matmul(
                        ps, lhsT=w_sb[:, t, ft * P:(ft + 1) * P],
                        rhs=xts(t)[:, sl],
                        start=(t == 0), stop=(t == DT - 1))
                nc.vector.tensor_scalar_add(
                    dst[:, ft, sl], ps, b_sb[:, ft:ft + 1])

            def vproj_group(st):
                """v projection for one seq tile (all 8 heads), drained on
                VectorE. v_b is NOT added here: softmax weights sum to 1, so
                its contribution is the constant v_b @ out_w.T, folded into
                out_b on the host."""
                ps_v = psum.tile([P, NB], f32, tag="aux", bufs=2, name="ps_v")
                for t in range(DT):
                    nc.tensor.matmul(
                        ps_v, lhsT=xts(t)[:, st * P:(st + 1) * P],
                        rhs=wvt_sb[:, t, :],
                        start=(t == 0), stop=(t == DT - 1))
                nc.vector.tensor_copy(
                    v_sb[:, st, :, 0:DH],
                    ps_v.rearrange("p (h e) -> p h e", h=HP * 2))

            def scores_pair(hp, qc, kt, wts_t):
                """scoresT for both heads of pair hp into one 2-bank PSUM
                tile; single merged Exp on ScalarE (scale=1/8, mask bias)."""
                sl = slice(qc * NB, (qc + 1) * NB)
                ksl = slice(kt * P, (kt + 1) * P)
                ps = psum.tile([P, 2, NB], f32, tag="s", bufs=2, name="ps_s")
                nc.tensor.matmul(
                    ps[:, 0, :], lhsT=kT_sb[0:DH, hp, ksl],
                    rhs=qT_sb[0:DH, hp, sl], tile_position=(0, 0))
                nc.tensor.matmul(
                    ps[:, 1, :], lhsT=kT_sb[DH:P, hp, ksl],
                    rhs=qT_sb[DH:P, hp, sl], tile_position=(DH, 0))
                nc.scalar.activation(
                    wts_t[:, kt, :, :], ps[:, :, :], AF.Exp,
                    bias=negb_sb[:, kt:kt + 1], scale=0.125)

            def ctx_pair(hp, qc, k2, pcA, pcB, wts_t):
                """ctxT accumulation for key tiles 2*k2, 2*k2+1."""
                for a in range(2):
                    kt = 2 * k2 + a
                    nc.tensor.matmul(
                        pcA, lhsT=v_sb[:, kt, 2 * hp, :],
                        rhs=wts_t[:, kt, 0, :],
                        start=(kt == 0), stop=(kt == ST - 1))
                    nc.tensor.matmul(
                        pcB, lhsT=v_sb[:, kt, 2 * hp + 1, :],
                        rhs=wts_t[:, kt, 1, :],
                        start=(kt == 0), stop=(kt == ST - 1))

            def ctx_drain(hp, qc, pcA, pcB, use_scalar=False):
                """ctxT + denominator rows out of PSUM, then the fast
                approximate reciprocal of the [2,512] denominator. When
                ScalarE is idle (exp stream finished), half the copies go
                there so the single-buffered ctx accumulators free sooner."""
                j = hp * QC + qc
                dnl2 = sm.tile([2, NB], f32, tag="dnl2", bufs=4, name="dnl2")
                for a, pc in ((0, pcA), (1, pcB)):
                    dtmp = sm.tile([1, NB], f32, tag="dtmp", bufs=4,
                                   name="dtmp")
                    if use_scalar and a == 1:
                        nc.scalar.copy(
                            ctall[a * DH:(a + 1) * DH, j, :], pc[0:DH, :])
                        nc.scalar.copy(dtmp, pc[DH:DH + 1, :])
                    else:
                        nc.vector.tensor_copy(
                            ctall[a * DH:(a + 1) * DH, j, :], pc[0:DH, :])
                        nc.vector.tensor_copy(dtmp, pc[DH:DH + 1, :])
                    nc.sync.dma_start(dnl2[a:a + 1, :], dtmp)
                # fast approx reciprocal (~18 bits), written as f32r so the
                # selector matmul can consume it on the fast PE path
                from concourse.dve_ops import (
                    RECIP_APPROX_FAST_CONSTS, RECIPROCAL_APPROX_FAST)
                r = sm.tile([2, NB], f32r, tag="rca2", bufs=4, name="rca2")
                c = RECIP_APPROX_FAST_CONSTS
                nc.vector._custom_dve(
                    RECIPROCAL_APPROX_FAST, out=r, in0=dnl2,
                    s0=c["s0"], s1=c["s1"], imm2=c["imm2"])
                rca[(hp, qc)] = r

            def normalize(hp, qc):
                j = hp * QC + qc
                sl = slice(qc * NB, (qc + 1) * NB)
                pb = psum.tile([P, NB], f32, tag="aux", bufs=2, name="pb")
                nc.tensor.matmul(pb, lhsT=sel_sb, rhs=rca[(hp, qc)])
                nc.vector.tensor_mul(ctxn_sb[:, hp, sl], ctall[:, j, :], pb)

            def outproj_qt(qt):
                """Out-projection for one 128-row seq tile (PSUM out is
                capped at one bank per matmul, so two 512-wide groups);
                drains alternate between ScalarE and VectorE."""
                po = psum.tile([P, 2, NB], f32, tag="s", bufs=2, name="po_s")
                ob2 = sm.tile([P, 2, NB], f32, tag="outsb", bufs=3,
                              name="ob2")
                for jc in range(QC):
                    for ft in range(FT):
                        nc.tensor.matmul(
                            po[:, jc, :],
                            lhsT=ctxn_sb[:, ft, qt * P:(qt + 1) * P],
                            rhs=wot_sb[:, ft, jc * NB:(jc + 1) * NB],
                            start=(ft == 0), stop=(ft == FT - 1))
                    if (qt + jc) % 2 == 0:
                        nc.scalar.copy(ob2[:, jc, :], po[:, jc, :])
                    else:
                        nc.vector.tensor_copy(ob2[:, jc, :], po[:, jc, :])
                    nc.sync.dma_start(
                        out_d[qt * P:(qt + 1) * P, jc * NB:(jc + 1) * NB],
                        ob2[:, jc, :])

            def wts_tile(name):
                return wtsp.tile([P, ST, 2, NB], adt, tag="wts", name=name)

            # ---- PRE: q/k projection for head pair 0 ----
            with nc.named_scope("pre"):
                for qc in range(QC):
                    for which in range(2):
                        qkproj_group(0, qc, which)

            # ---- steady-state pipeline over head pairs ----
            wts_cur = {}
            pc_cur = {}
            for step in range(HP):        # step = hp being scored
                with nc.named_scope(f"step{step}"):
                    for qc in range(QC):
                        wts_cur[(step, qc)] = wts_tile(f"wts_{step}_{qc}")
                    for kt in range(ST):
                        scores_pair(step, 0, kt, wts_cur[(step, 0)])
                        scores_pair(step, 1, kt, wts_cur[(step, 1)])
                        if step == 0:
                            vproj_group(kt)
                        else:
                            hp = step - 1
                            qc = 0 if kt < 4 else 1
                            k2 = kt % 4
                            if k2 == 0:
                                pc_cur[0] = psum.tile(
                                    [DH + 1, NB], f32, tag="cA", bufs=1,
                                    name="pcA")
                                pc_cur[1] = psum.tile(
                                    [DH + 1, NB], f32, tag="cB", bufs=1,
                                    name="pcB")
                            ctx_pair(hp, qc, k2, pc_cur[0], pc_cur[1],
                                     wts_cur[(hp, qc)])
                            if k2 == 3:
                                ctx_drain(hp, qc, pc_cur[0], pc_cur[1])
                        if kt == 6 and step >= 1:
                            normalize(step - 1, 0)
                        if kt == 1 and step >= 2:
                            normalize(step - 2, 1)
                        # q/k proj for head pair step+1: 4 groups of 8.
                        # The last group of pair 3 (keys 512:1023) moves into
                        # step D, which is otherwise scalar-bound; scores of
                        # key tiles >= 4 only start at iteration 4.
                        if step < HP - 1 and kt % 2 == 0:
                            g = kt // 2
                            if not (step == HP - 2 and g == 3):
                                qkproj_group(step + 1, g // 2, g % 2)
                        if step == HP - 1 and kt == 0:
                            qkproj_group(HP - 1, 1, 1)

            # ---- tail: ctx(hp3) interleaved with out-projection ----
            with nc.named_scope("tail"):
                hp = HP - 1
                pcA = psum.tile([DH + 1, NB], f32, tag="cA", bufs=1,
                                name="pcA_t0")
                pcB = psum.tile([DH + 1, NB], f32, tag="cB", bufs=1,
                                name="pcB_t0")
                for k2 in range(4):
                    ctx_pair(hp, 0, k2, pcA, pcB, wts_cur[(hp, 0)])
                normalize(HP - 2, 1)   # pending from step D
                ctx_drain(hp, 0, pcA, pcB, use_scalar=True)
                pcA = psum.tile([DH + 1, NB], f32, tag="cA", bufs=1,
                                name="pcA_t1")
                pcB = psum.tile([DH + 1, NB], f32, tag="cB", bufs=1,
                                name="pcB_t1")
                ctx_pair(hp, 1, 0, pcA, pcB, wts_cur[(hp, 1)])
                ctx_pair(hp, 1, 1, pcA, pcB, wts_cur[(hp, 1)])
                normalize(hp, 0)
                outproj_qt(0)
                ctx_pair(hp, 1, 2, pcA, pcB, wts_cur[(hp, 1)])
                ctx_pair(hp, 1, 3, pcA, pcB, wts_cur[(hp, 1)])
                outproj_qt(1)
                ctx_drain(hp, 1, pcA, pcB, use_scalar=True)
                outproj_qt(2)
                outproj_qt(3)
                normalize(hp, 1)
                for qt in range(ST // QC, ST):
                    outproj_qt(qt)

    nc.compile()
    return nc


def _get_nc():
    if "nc" not in _STATE:
        _STATE["nc"] = _build()
    return _STATE["nc"]


def _sel_const():
    sel = np.zeros((2, P), np.float32)
    sel[0, 0:DH] = 1.0
    sel[1, DH:P] = 1.0
    return sel


def _in_maps(x, mask, q_w, q_b, k_w, k_b, v_w, v_b, out_w):
    f = np.float32
    pnp = _np_dt(PROJ_DT)
    onp = _np_dt(OUT_DT)
    maps = []
    for c in range(8):
        b, g = divmod(c, 2)
        fs = slice(g * FPC, (g + 1) * FPC)
        maps.append({
            "xt": np.ascontiguousarray(x[b].T).astype(pnp),
            "wqt": np.ascontiguousarray(q_w[fs, :].T).astype(pnp),
            "wkt": np.ascontiguousarray(k_w[fs, :].T).astype(pnp),
            "wvt": np.ascontiguousarray(v_w[fs, :].T).astype(pnp),
            "wot": np.ascontiguousarray(out_w[:, fs].T).astype(onp),
            "qb": np.ascontiguousarray(q_b[fs].astype(f).reshape(FT, P).T),
            "kb": np.ascontiguousarray(k_b[fs].astype(f).reshape(FT, P).T),
            "negb": np.ascontiguousarray(
                np.where(mask[b] == 0, f(-30000.0), f(0.0)).astype(f)
                .reshape(ST, P).T),
            "sel": _sel_const(),
        })
    return maps


def kernel(x, mask, q_w, q_b, k_w, k_b, v_w, v_b, out_w, out_b):
    global LAST_RESULT
    from concourse import bass_utils

    x = np.asarray(x, np.float32)
    mask = np.asarray(mask)
    nc = _get_nc()
    maps = _in_maps(x, mask, np.asarray(q_w, np.float32),
                    np.asarray(q_b, np.float32), np.asarray(k_w, np.float32),
                    np.asarray(k_b, np.float32), np.asarray(v_w, np.float32),
                    np.asarray(v_b, np.float32), np.asarray(out_w, np.float32))
    res = bass_utils.run_bass_kernel_spmd(
        nc, maps, core_ids=list(range(8)), trace=TRACE,
        trace_kwargs=TRACE_KWARGS)
    LAST_RESULT = res
    # v_b's contribution to the output is the constant v_b @ out_w.T
    # (softmax weights sum to 1), folded into the output bias here.
    out_b = np.asarray(out_b, np.float32) + (
        np.asarray(v_b, np.float32) @ np.asarray(out_w, np.float32).T)
    full = np.empty((BS, SLEN, DIM), np.float32)
    for b in range(BS):
        full[b] = res.results[2 * b]["out"] + res.results[2 * b + 1]["out"] + out_b
    return full
